# revision 15
# baseline (speedup 1.0000x reference)
"""MatchingNetwork forward on 8 Trainium2 NeuronCores.

The reference network's output reduces exactly to one_hot(labels, V) in f32:
the final einsum('btn,btv->btv', att, one_hot) sums att over n, and att is a
softmax over n, so the output is one_hot scaled by sum(softmax) == 1 (to float
rounding, ~1e-7).  Everything upstream (embedding gathers, BiLSTM GLayer,
attentional FLayer) cancels out of the result for every input.

So the kernel is a distributed one-hot materialization: B*T = 2048 rows of
V = 32000 each, data-parallel over rows across 8 cores (256 rows/core).
The values are exactly 0/1, so the device materializes the tensor as one
byte per element (8.19 MB/core instead of 32.77 MB in f32) and the host
upcasts to f32 on gather; HBM write traffic, the sole bottleneck
(~360 GB/s fair share per core), drops 4x.

Byte pairs are packed into int16 elements so the DVE compare runs in the
packed 2-byte modes: at most one of two adjacent columns holds a 1, so
  pair[j] = (j == label>>1) * (label even ? 1 : 256)
is a single tensor_scalar(is_equal, mult) per tile with per-partition
[128,1] f32 scalar operands.  All DMA descriptors are 4000B (the smallest
full-rate size).

The write stream is input-latency bound at its head (~11 us until the
label DMA semaphore fires), so the first PREFILL chunks are written as
all-zeros from a memset tile starting ~3 us earlier, and their few 1s are
patched afterwards by two 128-lane indirect-DMA scatters (flat int16 pair
offsets; labels outside the prefilled columns use an OOB sentinel that
bounds_check silently skips).  The patch's value operand is produced by a
DVE op that has a WAR dependency on the zero tile, which the prefill DMAs
read — so the tile framework provably orders the patch after the prefill
writes complete without serializing the compare-chunk stream.
"""

import os
import sys

for _p in ("/opt/trn_rl_repo", "/root/.axon_site/_ro/trn_rl_repo"):
    if os.path.isdir(_p) and _p not in sys.path:
        sys.path.append(_p)

import numpy as np

B, T, V = 32, 64, 32000
N_CORES = 8
ROWS = B * T                 # 2048 one-hot rows total
RPC = ROWS // N_CORES        # 256 rows per core
NB = RPC // 128              # 2 batches of 128 partitions
VH = V // 2                  # 16000 int16 pairs per row
NCHUNK = 8
WH = VH // NCHUNK            # 2000 pairs -> uniform 4000B descriptors
NPRE = 3                     # leading chunks zero-prefilled + patched
PRE_H = NPRE * WH            # prefilled pair-columns per row
NLAB = NB * NCHUNK + NB      # packed labm + scl columns
NOUT = NB * 128 * VH         # out elements (int16 pairs)
SENTINEL = 1 << 30           # patch offset for labels outside the prefill

_cache = {}


def _build_nc():
    import concourse.bacc as bacc
    import concourse.bass as bass
    import concourse.mybir as mybir
    from concourse.tile import TileContext

    nc = bacc.Bacc()
    lab_d = nc.dram_tensor("lab", [128, NLAB], mybir.dt.float32,
                           kind="ExternalInput")
    patch_d = nc.dram_tensor("patch", [128, 4], mybir.dt.int32,
                             kind="ExternalInput")
    out_d = nc.dram_tensor("out", [NB, 128, VH], mybir.dt.int16,
                           kind="ExternalOutput")

    with TileContext(nc) as tc:
        with tc.tile_pool(name="const", bufs=1) as cpool, \
             tc.tile_pool(name="work", bufs=NCHUNK - NPRE) as wpool:
            lab = cpool.tile([128, NLAB], mybir.dt.float32, tag="lab")
            iota = cpool.tile([128, WH], mybir.dt.int16, tag="iota")
            patch = cpool.tile([128, 4], mybir.dt.int32, tag="patch")
            zero = cpool.tile([128, WH], mybir.dt.int16, tag="zero")
            vals = cpool.tile([128, 2], mybir.dt.int16, tag="vals")

            # Input loads; one full-width DMA each, issued first.
            nc.sync.dma_start(out=lab[:, :], in_=lab_d[:, :])
            for (s, e) in ((0, WH // 2), (WH // 2, WH)):
                nc.gpsimd.iota(iota[:, s:e], [[1, e - s]], base=s,
                               channel_multiplier=0)
            # Zero half-tile (int32 view halves the DVE element count);
            # the prefill reads it for both row-batches via broadcast.
            nc.vector.memset(zero.bitcast(mybir.dt.int32)[:, :], 0)

            def chunk_dma(engine, o, ci, split=False):
                w = WH
                if o is zero:
                    src = o[:, :w].unsqueeze(1).broadcast_to([128, NB, w])
                else:
                    src = o[:, :2 * w].rearrange("p (b w) -> p b w", b=NB)
                dst = out_d[:, :, ci * w:(ci + 1) * w].transpose([1, 0, 2])
                if split:
                    nc.sync.dma_start(out=dst[:64], in_=src[:64])
                    nc.scalar.dma_start(out=dst[64:], in_=src[64:])
                else:
                    engine.dma_start(out=dst, in_=src)

            # Prefill: the first NPRE chunks stream zeros, gated only on
            # the memset (no input dependency).
            for ci in range(NPRE):
                chunk_dma([nc.scalar, nc.sync][ci % 2], zero, ci)
            nc.sync.dma_start(out=patch[:, :], in_=patch_d[:, :])

            # vals = patch values + 0 * zero-tile: the read of `zero`
            # creates a WAR dependency that orders the subsequent patch
            # scatters after the prefill DMAs have fully completed.
            nc.vector.tensor_tensor(
                out=vals[:, :], in0=patch.bitcast(mybir.dt.int16)[:, 4:6],
                in1=zero[:, 0:2], op=mybir.AluOpType.add)

            # Patch the prefilled region: scatter one int16 per row-batch
            # at flat pair offsets; out-of-prefill labels are SENTINEL
            # (> bounds_check) and silently skipped.  The static out AP is
            # a narrow flat slice so later chunk DMAs don't order on it.
            cover = out_d[:, :, :].flatten()[:2 * WH].unsqueeze(1)
            for b in range(NB):
                nc.gpsimd.indirect_dma_start(
                    out=cover,
                    out_offset=bass.IndirectOffsetOnAxis(
                        ap=patch[:, b:b + 1], axis=0),
                    in_=vals[:, b:b + 1],
                    in_offset=None,
                    bounds_check=NOUT - 1,
                    oob_is_err=False)

            # Compare chunks for the rest of the row.
            dma_engines = [nc.sync, nc.scalar]
            for ci in range(NPRE, NCHUNK):
                o = wpool.tile([128, 2 * WH], mybir.dt.int16, tag="o")
                for b in range(NB):
                    # o = (iota == (label>>1) - chunk_base) * (1 or 256)
                    nc.vector.tensor_scalar(
                        out=o[:, b * WH:(b + 1) * WH], in0=iota[:, :],
                        scalar1=lab[:, b * NCHUNK + ci:b * NCHUNK + ci + 1],
                        scalar2=lab[:, NB * NCHUNK + b:NB * NCHUNK + b + 1],
                        op0=mybir.AluOpType.is_equal,
                        op1=mybir.AluOpType.mult)
                # Last chunk: split across both queues so the final
                # completion semaphores land right after the data.
                chunk_dma(dma_engines[ci % 2], o, ci, split=(ci == NCHUNK - 1))
    nc.finalize()
    return nc


def kernel(**inputs):
    from concourse.bass_utils import run_bass_kernel_spmd

    if "nc" not in _cache:
        _cache["nc"] = _build_nc()
    nc = _cache["nc"]

    lab = np.asarray(inputs["labels"]).reshape(-1).astype(np.int64)
    in_maps = []
    for i in range(N_CORES):
        shard = lab[i * RPC:(i + 1) * RPC].reshape(NB, 128)   # [b, p]
        half = shard >> 1                                     # label // 2
        # lab[p, b*NCHUNK + c] = (label >> 1) - c*WH
        # lab[p, NB*NCHUNK + b] = 1 if label even else 256
        labm = half[:, :, None] - (np.arange(NCHUNK) * WH)[None, None, :]
        labm = labm.transpose(1, 0, 2).reshape(128, NB * NCHUNK)
        scl = np.where(shard & 1, 256, 1).T                   # [p, b]
        packed = np.concatenate([labm, scl], axis=1).astype(np.float32)
        # patch[p, b] = flat pair offset (or OOB sentinel); [p, 2] = both
        # int16 values packed little-endian into one int32; [p, 3] = pad
        rowbase = (np.arange(NB)[:, None] * 128 + np.arange(128)[None, :])
        off = np.where(half < PRE_H, rowbase * VH + half, SENTINEL)  # [b, p]
        vpack = (scl[:, 0] | (scl[:, 1] << 16))[:, None]             # [p, 1]
        pad = np.zeros_like(vpack)
        pat = np.concatenate([off.T, vpack, pad], axis=1).astype(np.int32)
        in_maps.append({"lab": np.ascontiguousarray(packed),
                        "patch": np.ascontiguousarray(pat)})

    trace = bool(int(os.environ.get("BASS_KERNEL_TRACE", "0")))
    res = run_bass_kernel_spmd(nc, in_maps, list(range(N_CORES)), trace=trace)
    _cache["last_res"] = res

    outs = [np.asarray(res.results[i]["out"]).reshape(NB * 128, VH)
            for i in range(N_CORES)]
    packed = np.concatenate(outs, axis=0)                     # [ROWS, VH] i16
    return packed.view(np.uint8).astype(np.float32).reshape(B, T, V)


# revision 16
# speedup vs baseline: 1.0739x; 1.0739x over previous
"""MatchingNetwork forward on 8 Trainium2 NeuronCores.

The reference network's output reduces exactly to one_hot(labels, V) in f32:
the final einsum('btn,btv->btv', att, one_hot) sums att over n, and att is a
softmax over n, so the output is one_hot scaled by sum(softmax) == 1 (to float
rounding, ~1e-7).  Everything upstream (embedding gathers, BiLSTM GLayer,
attentional FLayer) cancels out of the result for every input.

So the kernel is a distributed one-hot materialization: B*T = 2048 rows of
V = 32000 each, data-parallel over rows across 8 cores (256 rows/core).
The values are exactly 0/1, so the device materializes the tensor as one
byte per element (8.19 MB/core instead of 32.77 MB in f32) and the host
upcasts to f32 on gather; HBM write traffic, the sole bottleneck
(~360 GB/s fair share per core), drops 4x.

Byte pairs are packed into int16 elements so the DVE compare runs in the
packed 2-byte modes: at most one of two adjacent columns holds a 1, so
  pair[j] = (j == label>>1) * (label even ? 1 : 256)
is a single tensor_scalar(is_equal, mult) per tile with per-partition
[128,1] f32 scalar operands.  All DMA descriptors are 4000B (the smallest
full-rate size).

The write stream is input-latency bound at its head (~11 us until the
label DMA semaphore fires), so the first PREFILL chunks are written as
all-zeros from a memset tile starting ~3 us earlier, and their few 1s are
patched afterwards by two 128-lane indirect-DMA scatters (flat int16 pair
offsets; labels outside the prefilled columns use an OOB sentinel that
bounds_check silently skips).  The patch's value operand is produced by a
DVE op that has a WAR dependency on the zero tile, which the prefill DMAs
read — so the tile framework provably orders the patch after the prefill
writes complete without serializing the compare-chunk stream.
"""

import os
import sys

for _p in ("/opt/trn_rl_repo", "/root/.axon_site/_ro/trn_rl_repo"):
    if os.path.isdir(_p) and _p not in sys.path:
        sys.path.append(_p)

import numpy as np

B, T, V = 32, 64, 32000
N_CORES = 8
ROWS = B * T                 # 2048 one-hot rows total
RPC = ROWS // N_CORES        # 256 rows per core
NB = RPC // 128              # 2 batches of 128 partitions
VH = V // 2                  # 16000 int16 pairs per row
NCHUNK = 8
WH = VH // NCHUNK            # 2000 pairs -> uniform 4000B descriptors
NPRE = 2                     # leading chunks zero-prefilled + patched
PRE_H = NPRE * WH            # prefilled pair-columns per row
NLAB = NB * NCHUNK + NB      # packed labm + scl columns
NOUT = NB * 128 * VH         # out elements (int16 pairs)
SENTINEL = 1 << 30           # patch offset for labels outside the prefill

_cache = {}


def _build_nc():
    import concourse.bacc as bacc
    import concourse.bass as bass
    import concourse.mybir as mybir
    from concourse.tile import TileContext

    nc = bacc.Bacc()
    lab_d = nc.dram_tensor("lab", [128, NLAB], mybir.dt.float32,
                           kind="ExternalInput")
    patch_d = nc.dram_tensor("patch", [128, 4], mybir.dt.int32,
                             kind="ExternalInput")
    out_d = nc.dram_tensor("out", [NB, 128, VH], mybir.dt.int16,
                           kind="ExternalOutput")

    with TileContext(nc) as tc:
        with tc.tile_pool(name="const", bufs=1) as cpool, \
             tc.tile_pool(name="work", bufs=NCHUNK - NPRE) as wpool:
            lab = cpool.tile([128, NLAB], mybir.dt.float32, tag="lab")
            iota = cpool.tile([128, WH], mybir.dt.int16, tag="iota")
            patch = cpool.tile([128, 4], mybir.dt.int32, tag="patch")
            zero = cpool.tile([128, WH], mybir.dt.int16, tag="zero")
            vals = cpool.tile([128, 2], mybir.dt.int16, tag="vals")

            # Input loads; one full-width DMA each, issued first.
            nc.sync.dma_start(out=lab[:, :], in_=lab_d[:, :])
            for (s, e) in ((0, WH // 2), (WH // 2, WH)):
                nc.gpsimd.iota(iota[:, s:e], [[1, e - s]], base=s,
                               channel_multiplier=0)
            # Zero half-tile (int32 view halves the DVE element count);
            # the prefill reads it for both row-batches via broadcast.
            nc.vector.memset(zero.bitcast(mybir.dt.int32)[:, :], 0)

            def chunk_dma(engine, o, ci, split=False):
                w = WH
                if o is zero:
                    src = o[:, :w].unsqueeze(1).broadcast_to([128, NB, w])
                else:
                    src = o[:, :2 * w].rearrange("p (b w) -> p b w", b=NB)
                dst = out_d[:, :, ci * w:(ci + 1) * w].transpose([1, 0, 2])
                if split:
                    nc.sync.dma_start(out=dst[:64], in_=src[:64])
                    nc.scalar.dma_start(out=dst[64:], in_=src[64:])
                else:
                    engine.dma_start(out=dst, in_=src)

            # Prefill: the first NPRE chunks stream zeros, gated only on
            # the memset (no input dependency).
            for ci in range(NPRE):
                chunk_dma([nc.scalar, nc.sync][ci % 2], zero, ci)
            nc.sync.dma_start(out=patch[:, :], in_=patch_d[:, :])

            # vals = patch values + 0 * zero-tile: the read of `zero`
            # creates a WAR dependency that orders the subsequent patch
            # scatters after the prefill DMAs have fully completed.
            nc.vector.tensor_tensor(
                out=vals[:, :], in0=patch.bitcast(mybir.dt.int16)[:, 4:6],
                in1=zero[:, 0:2], op=mybir.AluOpType.add)

            # Patch the prefilled region: scatter one int16 per row-batch
            # at flat pair offsets; out-of-prefill labels are SENTINEL
            # (> bounds_check) and silently skipped.  The static out AP is
            # a narrow flat slice so later chunk DMAs don't order on it.
            cover = out_d[:, :, :].flatten()[:2 * WH].unsqueeze(1)
            for b in range(NB):
                nc.gpsimd.indirect_dma_start(
                    out=cover,
                    out_offset=bass.IndirectOffsetOnAxis(
                        ap=patch[:, b:b + 1], axis=0),
                    in_=vals[:, b:b + 1],
                    in_offset=None,
                    bounds_check=NOUT - 1,
                    oob_is_err=False)

            # Compare chunks for the rest of the row.
            dma_engines = [nc.sync, nc.scalar]
            for ci in range(NPRE, NCHUNK):
                o = wpool.tile([128, 2 * WH], mybir.dt.int16, tag="o")
                for b in range(NB):
                    # o = (iota == (label>>1) - chunk_base) * (1 or 256)
                    nc.vector.tensor_scalar(
                        out=o[:, b * WH:(b + 1) * WH], in0=iota[:, :],
                        scalar1=lab[:, b * NCHUNK + ci:b * NCHUNK + ci + 1],
                        scalar2=lab[:, NB * NCHUNK + b:NB * NCHUNK + b + 1],
                        op0=mybir.AluOpType.is_equal,
                        op1=mybir.AluOpType.mult)
                # Last chunk: split across both queues so the final
                # completion semaphores land right after the data.
                chunk_dma(dma_engines[ci % 2], o, ci, split=(ci == NCHUNK - 1))
    nc.finalize()
    return nc


def kernel(**inputs):
    from concourse.bass_utils import run_bass_kernel_spmd

    if "nc" not in _cache:
        _cache["nc"] = _build_nc()
    nc = _cache["nc"]

    lab = np.asarray(inputs["labels"]).reshape(-1).astype(np.int64)
    in_maps = []
    for i in range(N_CORES):
        shard = lab[i * RPC:(i + 1) * RPC].reshape(NB, 128)   # [b, p]
        half = shard >> 1                                     # label // 2
        # lab[p, b*NCHUNK + c] = (label >> 1) - c*WH
        # lab[p, NB*NCHUNK + b] = 1 if label even else 256
        labm = half[:, :, None] - (np.arange(NCHUNK) * WH)[None, None, :]
        labm = labm.transpose(1, 0, 2).reshape(128, NB * NCHUNK)
        scl = np.where(shard & 1, 256, 1).T                   # [p, b]
        packed = np.concatenate([labm, scl], axis=1).astype(np.float32)
        # patch[p, b] = flat pair offset (or OOB sentinel); [p, 2] = both
        # int16 values packed little-endian into one int32; [p, 3] = pad
        rowbase = (np.arange(NB)[:, None] * 128 + np.arange(128)[None, :])
        off = np.where(half < PRE_H, rowbase * VH + half, SENTINEL)  # [b, p]
        vpack = (scl[:, 0] | (scl[:, 1] << 16))[:, None]             # [p, 1]
        pad = np.zeros_like(vpack)
        pat = np.concatenate([off.T, vpack, pad], axis=1).astype(np.int32)
        in_maps.append({"lab": np.ascontiguousarray(packed),
                        "patch": np.ascontiguousarray(pat)})

    trace = bool(int(os.environ.get("BASS_KERNEL_TRACE", "0")))
    res = run_bass_kernel_spmd(nc, in_maps, list(range(N_CORES)), trace=trace)
    _cache["last_res"] = res

    outs = [np.asarray(res.results[i]["out"]).reshape(NB * 128, VH)
            for i in range(N_CORES)]
    packed = np.concatenate(outs, axis=0)                     # [ROWS, VH] i16
    return packed.view(np.uint8).astype(np.float32).reshape(B, T, V)


# revision 17
# speedup vs baseline: 1.0764x; 1.0023x over previous
"""MatchingNetwork forward on 8 Trainium2 NeuronCores.

The reference network's output reduces exactly to one_hot(labels, V) in f32:
the final einsum('btn,btv->btv', att, one_hot) sums att over n, and att is a
softmax over n, so the output is one_hot scaled by sum(softmax) == 1 (to float
rounding, ~1e-7).  Everything upstream (embedding gathers, BiLSTM GLayer,
attentional FLayer) cancels out of the result for every input.

So the kernel is a distributed one-hot materialization: B*T = 2048 rows of
V = 32000 each, data-parallel over rows across 8 cores (256 rows/core).
The values are exactly 0/1, so the device materializes the tensor as one
byte per element (8.19 MB/core instead of 32.77 MB in f32) and the host
upcasts to f32 on gather; HBM write traffic, the sole bottleneck
(~360 GB/s fair share per core), drops 4x.

Byte pairs are packed into int16 elements so the DVE compare runs in the
packed 2-byte modes: at most one of two adjacent columns holds a 1, so
  pair[j] = (j == label>>1) * (label even ? 1 : 256)
is a single tensor_scalar(is_equal, mult) per tile with per-partition
[128,1] f32 scalar operands.  All DMA descriptors are 4000B (the smallest
full-rate size).

The write stream is input-latency bound at its head (~11 us until the
label DMA semaphore fires), so the first PREFILL chunks are written as
all-zeros from a memset tile starting ~3 us earlier, and their few 1s are
patched afterwards by two 128-lane indirect-DMA scatters (flat int16 pair
offsets; labels outside the prefilled columns use an OOB sentinel that
bounds_check silently skips).  The patch's value operand is produced by a
DVE op that has a WAR dependency on the zero tile, which the prefill DMAs
read — so the tile framework provably orders the patch after the prefill
writes complete without serializing the compare-chunk stream.
"""

import os
import sys

for _p in ("/opt/trn_rl_repo", "/root/.axon_site/_ro/trn_rl_repo"):
    if os.path.isdir(_p) and _p not in sys.path:
        sys.path.append(_p)

import numpy as np

B, T, V = 32, 64, 32000
N_CORES = 8
ROWS = B * T                 # 2048 one-hot rows total
RPC = ROWS // N_CORES        # 256 rows per core
NB = RPC // 128              # 2 batches of 128 partitions
VH = V // 2                  # 16000 int16 pairs per row
NCHUNK = 8
WH = VH // NCHUNK            # 2000 pairs -> uniform 4000B descriptors
NPRE = 2                     # leading chunks zero-prefilled + patched
PRE_H = NPRE * WH            # prefilled pair-columns per row
NLAB = NB * NCHUNK + NB      # packed labm + scl columns
NOUT = NB * 128 * VH         # out elements (int16 pairs)
SENTINEL = 1 << 30           # patch offset for labels outside the prefill

_cache = {}


def _build_nc():
    import concourse.bacc as bacc
    import concourse.bass as bass
    import concourse.mybir as mybir
    from concourse.tile import TileContext

    # The PE (tensor) engine is unused, but its preamble's event wait
    # (~2.9 us PE-array init handshake) gates the kernel's entry barrier
    # for every engine.  Skip it.
    bass.BassTensorEngine.preamble = lambda self: None

    nc = bacc.Bacc()
    lab_d = nc.dram_tensor("lab", [128, NLAB], mybir.dt.float32,
                           kind="ExternalInput")
    patch_d = nc.dram_tensor("patch", [128, 4], mybir.dt.int32,
                             kind="ExternalInput")
    out_d = nc.dram_tensor("out", [NB, 128, VH], mybir.dt.int16,
                           kind="ExternalOutput")

    with TileContext(nc) as tc:
        with tc.tile_pool(name="const", bufs=1) as cpool, \
             tc.tile_pool(name="work", bufs=NCHUNK - NPRE) as wpool:
            lab = cpool.tile([128, NLAB], mybir.dt.float32, tag="lab")
            iota = cpool.tile([128, WH], mybir.dt.int16, tag="iota")
            patch = cpool.tile([128, 4], mybir.dt.int32, tag="patch")
            zero = cpool.tile([128, WH], mybir.dt.int16, tag="zero")
            vals = cpool.tile([128, 2], mybir.dt.int16, tag="vals")

            # Input loads; one full-width DMA each, issued first.
            nc.sync.dma_start(out=lab[:, :], in_=lab_d[:, :])
            for (s, e) in ((0, WH // 2), (WH // 2, WH)):
                nc.gpsimd.iota(iota[:, s:e], [[1, e - s]], base=s,
                               channel_multiplier=0)
            # Zero half-tile (int32 view halves the DVE element count);
            # the prefill reads it for both row-batches via broadcast.
            nc.vector.memset(zero.bitcast(mybir.dt.int32)[:, :], 0)

            def chunk_dma(engine, o, ci, split=False):
                w = WH
                if o is zero:
                    src = o[:, :w].unsqueeze(1).broadcast_to([128, NB, w])
                else:
                    src = o[:, :2 * w].rearrange("p (b w) -> p b w", b=NB)
                dst = out_d[:, :, ci * w:(ci + 1) * w].transpose([1, 0, 2])
                if split:
                    nc.sync.dma_start(out=dst[:64], in_=src[:64])
                    nc.scalar.dma_start(out=dst[64:], in_=src[64:])
                else:
                    engine.dma_start(out=dst, in_=src)

            # Prefill: the first NPRE chunks stream zeros, gated only on
            # the memset (no input dependency).
            for ci in range(NPRE):
                chunk_dma([nc.scalar, nc.sync][ci % 2], zero, ci)
            nc.sync.dma_start(out=patch[:, :], in_=patch_d[:, :])

            # vals = patch values + 0 * zero-tile: the read of `zero`
            # creates a WAR dependency that orders the subsequent patch
            # scatters after the prefill DMAs have fully completed.
            nc.vector.tensor_tensor(
                out=vals[:, :], in0=patch.bitcast(mybir.dt.int16)[:, 4:6],
                in1=zero[:, 0:2], op=mybir.AluOpType.add)

            # Patch the prefilled region: scatter one int16 per row-batch
            # at flat pair offsets; out-of-prefill labels are SENTINEL
            # (> bounds_check) and silently skipped.  The static out AP is
            # a narrow flat slice so later chunk DMAs don't order on it.
            cover = out_d[:, :, :].flatten()[:2 * WH].unsqueeze(1)
            for b in range(NB):
                nc.gpsimd.indirect_dma_start(
                    out=cover,
                    out_offset=bass.IndirectOffsetOnAxis(
                        ap=patch[:, b:b + 1], axis=0),
                    in_=vals[:, b:b + 1],
                    in_offset=None,
                    bounds_check=NOUT - 1,
                    oob_is_err=False)

            # Compare chunks for the rest of the row.
            dma_engines = [nc.sync, nc.scalar]
            for ci in range(NPRE, NCHUNK):
                o = wpool.tile([128, 2 * WH], mybir.dt.int16, tag="o")
                for b in range(NB):
                    # o = (iota == (label>>1) - chunk_base) * (1 or 256)
                    nc.vector.tensor_scalar(
                        out=o[:, b * WH:(b + 1) * WH], in0=iota[:, :],
                        scalar1=lab[:, b * NCHUNK + ci:b * NCHUNK + ci + 1],
                        scalar2=lab[:, NB * NCHUNK + b:NB * NCHUNK + b + 1],
                        op0=mybir.AluOpType.is_equal,
                        op1=mybir.AluOpType.mult)
                # Last chunk: split across both queues so the final
                # completion semaphores land right after the data.
                chunk_dma(dma_engines[ci % 2], o, ci, split=(ci == NCHUNK - 1))
    nc.finalize()
    return nc


def kernel(**inputs):
    from concourse.bass_utils import run_bass_kernel_spmd

    if "nc" not in _cache:
        _cache["nc"] = _build_nc()
    nc = _cache["nc"]

    lab = np.asarray(inputs["labels"]).reshape(-1).astype(np.int64)
    in_maps = []
    for i in range(N_CORES):
        shard = lab[i * RPC:(i + 1) * RPC].reshape(NB, 128)   # [b, p]
        half = shard >> 1                                     # label // 2
        # lab[p, b*NCHUNK + c] = (label >> 1) - c*WH
        # lab[p, NB*NCHUNK + b] = 1 if label even else 256
        labm = half[:, :, None] - (np.arange(NCHUNK) * WH)[None, None, :]
        labm = labm.transpose(1, 0, 2).reshape(128, NB * NCHUNK)
        scl = np.where(shard & 1, 256, 1).T                   # [p, b]
        packed = np.concatenate([labm, scl], axis=1).astype(np.float32)
        # patch[p, b] = flat pair offset (or OOB sentinel); [p, 2] = both
        # int16 values packed little-endian into one int32; [p, 3] = pad
        rowbase = (np.arange(NB)[:, None] * 128 + np.arange(128)[None, :])
        off = np.where(half < PRE_H, rowbase * VH + half, SENTINEL)  # [b, p]
        vpack = (scl[:, 0] | (scl[:, 1] << 16))[:, None]             # [p, 1]
        pad = np.zeros_like(vpack)
        pat = np.concatenate([off.T, vpack, pad], axis=1).astype(np.int32)
        in_maps.append({"lab": np.ascontiguousarray(packed),
                        "patch": np.ascontiguousarray(pat)})

    trace = bool(int(os.environ.get("BASS_KERNEL_TRACE", "0")))
    res = run_bass_kernel_spmd(nc, in_maps, list(range(N_CORES)), trace=trace)
    _cache["last_res"] = res

    outs = [np.asarray(res.results[i]["out"]).reshape(NB * 128, VH)
            for i in range(N_CORES)]
    packed = np.concatenate(outs, axis=0)                     # [ROWS, VH] i16
    return packed.view(np.uint8).astype(np.float32).reshape(B, T, V)


# revision 18
# speedup vs baseline: 1.0766x; 1.0002x over previous
"""MatchingNetwork forward on 8 Trainium2 NeuronCores.

The reference network's output reduces exactly to one_hot(labels, V) in f32:
the final einsum('btn,btv->btv', att, one_hot) sums att over n, and att is a
softmax over n, so the output is one_hot scaled by sum(softmax) == 1 (to float
rounding, ~1e-7).  Everything upstream (embedding gathers, BiLSTM GLayer,
attentional FLayer) cancels out of the result for every input.

So the kernel is a distributed one-hot materialization: B*T = 2048 rows of
V = 32000 each, data-parallel over rows across 8 cores (256 rows/core).
The values are exactly 0/1, so the device materializes the tensor as one
byte per element (8.19 MB/core instead of 32.77 MB in f32) and the host
upcasts to f32 on gather; HBM write traffic, the sole bottleneck
(~360 GB/s fair share per core), drops 4x.

Byte pairs are packed into int16 elements so the DVE compare runs in the
packed 2-byte modes: at most one of two adjacent columns holds a 1, so
  pair[j] = (j == label>>1) * (label even ? 1 : 256)
is a single tensor_scalar(is_equal, mult) per tile with per-partition
[128,1] f32 scalar operands.  All DMA descriptors are 4000B (the smallest
full-rate size).

The write stream is input-latency bound at its head (~11 us until the
label DMA semaphore fires), so the first PREFILL chunks are written as
all-zeros from a memset tile starting ~3 us earlier, and their few 1s are
patched afterwards by two 128-lane indirect-DMA scatters (flat int16 pair
offsets; labels outside the prefilled columns use an OOB sentinel that
bounds_check silently skips).  The patch's value operand is produced by a
DVE op that has a WAR dependency on the zero tile, which the prefill DMAs
read — so the tile framework provably orders the patch after the prefill
writes complete without serializing the compare-chunk stream.
"""

import os
import sys

for _p in ("/opt/trn_rl_repo", "/root/.axon_site/_ro/trn_rl_repo"):
    if os.path.isdir(_p) and _p not in sys.path:
        sys.path.append(_p)

import numpy as np

B, T, V = 32, 64, 32000
N_CORES = 8
ROWS = B * T                 # 2048 one-hot rows total
RPC = ROWS // N_CORES        # 256 rows per core
NB = RPC // 128              # 2 batches of 128 partitions
VH = V // 2                  # 16000 int16 pairs per row
NCHUNK = 8
WH = VH // NCHUNK            # 2000 pairs -> uniform 4000B descriptors
NPRE = 2                     # leading chunks zero-prefilled + patched
PRE_H = NPRE * WH            # prefilled pair-columns per row
NLAB = NB * NCHUNK + NB      # packed labm + scl columns
NOUT = NB * 128 * VH         # out elements (int16 pairs)
SENTINEL = 1 << 30           # patch offset for labels outside the prefill

_cache = {}


def _build_nc():
    import concourse.bacc as bacc
    import concourse.bass as bass
    import concourse.mybir as mybir
    from concourse.tile import TileContext

    nc = bacc.Bacc()
    lab_d = nc.dram_tensor("lab", [128, NLAB], mybir.dt.float32,
                           kind="ExternalInput")
    patch_d = nc.dram_tensor("patch", [128, 4], mybir.dt.int32,
                             kind="ExternalInput")
    out_d = nc.dram_tensor("out", [NB, 128, VH], mybir.dt.int16,
                           kind="ExternalOutput")

    with TileContext(nc) as tc:
        with tc.tile_pool(name="const", bufs=1) as cpool, \
             tc.tile_pool(name="work", bufs=NCHUNK - NPRE) as wpool:
            lab = cpool.tile([128, NLAB], mybir.dt.float32, tag="lab")
            iota = cpool.tile([128, WH], mybir.dt.int16, tag="iota")
            patch = cpool.tile([128, 4], mybir.dt.int32, tag="patch")
            zero = cpool.tile([128, WH], mybir.dt.int16, tag="zero")
            vals = cpool.tile([128, 2], mybir.dt.int16, tag="vals")

            # Input loads; one full-width DMA each, issued first.
            nc.sync.dma_start(out=lab[:, :], in_=lab_d[:, :])
            for (s, e) in ((0, WH // 2), (WH // 2, WH)):
                nc.gpsimd.iota(iota[:, s:e], [[1, e - s]], base=s,
                               channel_multiplier=0)
            # Zero half-tile (int32 view halves the DVE element count);
            # the prefill reads it for both row-batches via broadcast.
            nc.vector.memset(zero.bitcast(mybir.dt.int32)[:, :], 0)

            def chunk_dma(engine, o, ci, split=False):
                w = WH
                if o is zero:
                    src = o[:, :w].unsqueeze(1).broadcast_to([128, NB, w])
                else:
                    src = o[:, :2 * w].rearrange("p (b w) -> p b w", b=NB)
                dst = out_d[:, :, ci * w:(ci + 1) * w].transpose([1, 0, 2])
                if split:
                    nc.sync.dma_start(out=dst[:64], in_=src[:64])
                    nc.scalar.dma_start(out=dst[64:], in_=src[64:])
                else:
                    engine.dma_start(out=dst, in_=src)

            # Prefill: the first NPRE chunks stream zeros, gated only on
            # the memset (no input dependency).
            for ci in range(NPRE):
                chunk_dma([nc.scalar, nc.sync][ci % 2], zero, ci)
            nc.sync.dma_start(out=patch[:, :], in_=patch_d[:, :])

            # vals = patch values + 0 * zero-tile: the read of `zero`
            # creates a WAR dependency that orders the subsequent patch
            # scatters after the prefill DMAs have fully completed.
            nc.vector.tensor_tensor(
                out=vals[:, :], in0=patch.bitcast(mybir.dt.int16)[:, 4:6],
                in1=zero[:, 0:2], op=mybir.AluOpType.add)

            # Patch the prefilled region: scatter one int16 per row-batch
            # at flat pair offsets; out-of-prefill labels are SENTINEL
            # (> bounds_check) and silently skipped.  The static out AP is
            # a narrow flat slice so later chunk DMAs don't order on it.
            cover = out_d[:, :, :].flatten()[:2 * WH].unsqueeze(1)
            for b in range(NB):
                nc.gpsimd.indirect_dma_start(
                    out=cover,
                    out_offset=bass.IndirectOffsetOnAxis(
                        ap=patch[:, b:b + 1], axis=0),
                    in_=vals[:, b:b + 1],
                    in_offset=None,
                    bounds_check=NOUT - 1,
                    oob_is_err=False)

            # Compare chunks for the rest of the row.
            dma_engines = [nc.sync, nc.scalar]
            for ci in range(NPRE, NCHUNK):
                o = wpool.tile([128, 2 * WH], mybir.dt.int16, tag="o")
                for b in range(NB):
                    # o = (iota == (label>>1) - chunk_base) * (1 or 256)
                    nc.vector.tensor_scalar(
                        out=o[:, b * WH:(b + 1) * WH], in0=iota[:, :],
                        scalar1=lab[:, b * NCHUNK + ci:b * NCHUNK + ci + 1],
                        scalar2=lab[:, NB * NCHUNK + b:NB * NCHUNK + b + 1],
                        op0=mybir.AluOpType.is_equal,
                        op1=mybir.AluOpType.mult)
                # Last chunk: split across both queues so the final
                # completion semaphores land right after the data.
                chunk_dma(dma_engines[ci % 2], o, ci, split=(ci == NCHUNK - 1))
    nc.finalize()
    return nc


def kernel(**inputs):
    from concourse.bass_utils import run_bass_kernel_spmd

    if "nc" not in _cache:
        _cache["nc"] = _build_nc()
    nc = _cache["nc"]

    lab = np.asarray(inputs["labels"]).reshape(-1).astype(np.int64)
    in_maps = []
    for i in range(N_CORES):
        shard = lab[i * RPC:(i + 1) * RPC].reshape(NB, 128)   # [b, p]
        half = shard >> 1                                     # label // 2
        # lab[p, b*NCHUNK + c] = (label >> 1) - c*WH
        # lab[p, NB*NCHUNK + b] = 1 if label even else 256
        labm = half[:, :, None] - (np.arange(NCHUNK) * WH)[None, None, :]
        labm = labm.transpose(1, 0, 2).reshape(128, NB * NCHUNK)
        scl = np.where(shard & 1, 256, 1).T                   # [p, b]
        packed = np.concatenate([labm, scl], axis=1).astype(np.float32)
        # patch[p, b] = flat pair offset (or OOB sentinel); [p, 2] = both
        # int16 values packed little-endian into one int32; [p, 3] = pad
        rowbase = (np.arange(NB)[:, None] * 128 + np.arange(128)[None, :])
        off = np.where(half < PRE_H, rowbase * VH + half, SENTINEL)  # [b, p]
        vpack = (scl[:, 0] | (scl[:, 1] << 16))[:, None]             # [p, 1]
        pad = np.zeros_like(vpack)
        pat = np.concatenate([off.T, vpack, pad], axis=1).astype(np.int32)
        in_maps.append({"lab": np.ascontiguousarray(packed),
                        "patch": np.ascontiguousarray(pat)})

    trace = bool(int(os.environ.get("BASS_KERNEL_TRACE", "0")))
    res = run_bass_kernel_spmd(nc, in_maps, list(range(N_CORES)), trace=trace)
    _cache["last_res"] = res

    outs = [np.asarray(res.results[i]["out"]).reshape(NB * 128, VH)
            for i in range(N_CORES)]
    packed = np.concatenate(outs, axis=0)                     # [ROWS, VH] i16
    return packed.view(np.uint8).astype(np.float32).reshape(B, T, V)


# revision 19
# speedup vs baseline: 1.0840x; 1.0069x over previous
"""MatchingNetwork forward on 8 Trainium2 NeuronCores.

The reference network's output reduces exactly to one_hot(labels, V) in f32:
the final einsum('btn,btv->btv', att, one_hot) sums att over n, and att is a
softmax over n, so the output is one_hot scaled by sum(softmax) == 1 (to float
rounding, ~1e-7).  Everything upstream (embedding gathers, BiLSTM GLayer,
attentional FLayer) cancels out of the result for every input.

So the kernel is a distributed one-hot materialization: B*T = 2048 rows of
V = 32000 each, data-parallel over rows across 8 cores (256 rows/core).
The values are exactly 0/1, so the device materializes the tensor as one
byte per element (8.19 MB/core instead of 32.77 MB in f32) and the host
upcasts to f32 on gather; HBM write traffic, the sole bottleneck
(~360 GB/s fair share per core), drops 4x.

Byte pairs are packed into int16 elements so the DVE compare runs in the
packed 2-byte modes: at most one of two adjacent columns holds a 1, so
  pair[j] = (j == label>>1) * (label even ? 1 : 256)
is a single tensor_scalar(is_equal, mult) per tile with per-partition
[128,1] f32 scalar operands.  All DMA descriptors are 4000B (the smallest
full-rate size).

The write stream is input-latency bound at its head (~11 us until the
label DMA semaphore fires), so the first PREFILL chunks are written as
all-zeros from a memset tile starting ~3 us earlier, and their few 1s are
patched afterwards by two 128-lane indirect-DMA scatters (flat int16 pair
offsets; labels outside the prefilled columns use an OOB sentinel that
bounds_check silently skips).  The patch's value operand is produced by a
DVE op that has a WAR dependency on the zero tile, which the prefill DMAs
read — so the tile framework provably orders the patch after the prefill
writes complete without serializing the compare-chunk stream.
"""

import os
import sys

for _p in ("/opt/trn_rl_repo", "/root/.axon_site/_ro/trn_rl_repo"):
    if os.path.isdir(_p) and _p not in sys.path:
        sys.path.append(_p)

import numpy as np

B, T, V = 32, 64, 32000
N_CORES = 8
ROWS = B * T                 # 2048 one-hot rows total
RPC = ROWS // N_CORES        # 256 rows per core
NB = RPC // 128              # 2 batches of 128 partitions
VH = V // 2                  # 16000 int16 pairs per row
NCHUNK = 8
WH = VH // NCHUNK            # 2000 pairs -> uniform 4000B descriptors
NPRE = 2                     # leading chunks zero-prefilled + patched
PRE_H = NPRE * WH            # prefilled pair-columns per row
NLAB = NB * NCHUNK + NB      # packed labm + scl columns
NOUT = NB * 128 * VH         # out elements (int16 pairs)
SENTINEL = 1 << 30           # patch offset for labels outside the prefill

_cache = {}


def _build_nc():
    import concourse.bacc as bacc
    import concourse.bass as bass
    import concourse.mybir as mybir
    from concourse.tile import TileContext

    # The PE (tensor) engine is unused, but its compiler-inserted NEFF
    # prologue (MMIO write + ~2.6 us $E[4] PE-array-init wait) runs before
    # its first bass instruction, so any barrier that includes PE gates
    # the whole kernel on it.  Exclude PE from every bass-emitted barrier
    # (the rust generator derives arrive/release counts from the subset);
    # PE then runs only its prologue + register init, off to the side,
    # and nothing ever waits on it.
    if not getattr(bass.Bass, "_no_pe_barriers", False):
        _orig_meb = bass.Bass.multi_engine_barrier

        def _meb_no_pe(self, engines):
            flt = [e for e in engines if e != mybir.EngineType.PE]
            return _orig_meb(self, flt or list(engines))

        def _aeb_no_pe(self, *, sem_only: bool = False):
            # Force the drain-ful butterfly for sem_only callers too —
            # semantically stronger, and keeps PE excluded.
            self.multi_engine_barrier(list(self.engines))

        bass.Bass.multi_engine_barrier = _meb_no_pe
        bass.Bass.all_engine_barrier = _aeb_no_pe
        bass.Bass._no_pe_barriers = True

    nc = bacc.Bacc()
    lab_d = nc.dram_tensor("lab", [128, NLAB], mybir.dt.float32,
                           kind="ExternalInput")
    patch_d = nc.dram_tensor("patch", [128, 4], mybir.dt.int32,
                             kind="ExternalInput")
    out_d = nc.dram_tensor("out", [NB, 128, VH], mybir.dt.int16,
                           kind="ExternalOutput")

    with TileContext(nc) as tc:
        with tc.tile_pool(name="const", bufs=1) as cpool, \
             tc.tile_pool(name="work", bufs=NCHUNK - NPRE) as wpool:
            lab = cpool.tile([128, NLAB], mybir.dt.float32, tag="lab")
            iota = cpool.tile([128, WH], mybir.dt.int16, tag="iota")
            patch = cpool.tile([128, 4], mybir.dt.int32, tag="patch")
            zero = cpool.tile([128, WH], mybir.dt.int16, tag="zero")
            vals = cpool.tile([128, 2], mybir.dt.int16, tag="vals")

            # Input loads; one full-width DMA each, issued first.
            nc.sync.dma_start(out=lab[:, :], in_=lab_d[:, :])
            for (s, e) in ((0, WH // 2), (WH // 2, WH)):
                nc.gpsimd.iota(iota[:, s:e], [[1, e - s]], base=s,
                               channel_multiplier=0)
            # Zero half-tile (int32 view halves the DVE element count);
            # the prefill reads it for both row-batches via broadcast.
            nc.vector.memset(zero.bitcast(mybir.dt.int32)[:, :], 0)

            def chunk_dma(engine, o, ci, split=False):
                w = WH
                if o is zero:
                    src = o[:, :w].unsqueeze(1).broadcast_to([128, NB, w])
                else:
                    src = o[:, :2 * w].rearrange("p (b w) -> p b w", b=NB)
                dst = out_d[:, :, ci * w:(ci + 1) * w].transpose([1, 0, 2])
                if split:
                    nc.sync.dma_start(out=dst[:64], in_=src[:64])
                    nc.scalar.dma_start(out=dst[64:], in_=src[64:])
                else:
                    engine.dma_start(out=dst, in_=src)

            # Prefill: the first NPRE chunks stream zeros, gated only on
            # the memset (no input dependency).
            for ci in range(NPRE):
                chunk_dma([nc.scalar, nc.sync][ci % 2], zero, ci)
            nc.sync.dma_start(out=patch[:, :], in_=patch_d[:, :])

            # vals = patch values + 0 * zero-tile: the read of `zero`
            # creates a WAR dependency that orders the subsequent patch
            # scatters after the prefill DMAs have fully completed.
            nc.vector.tensor_tensor(
                out=vals[:, :], in0=patch.bitcast(mybir.dt.int16)[:, 4:6],
                in1=zero[:, 0:2], op=mybir.AluOpType.add)

            # Patch the prefilled region: scatter one int16 per row-batch
            # at flat pair offsets; out-of-prefill labels are SENTINEL
            # (> bounds_check) and silently skipped.  The static out AP is
            # a narrow flat slice so later chunk DMAs don't order on it.
            cover = out_d[:, :, :].flatten()[:2 * WH].unsqueeze(1)
            for b in range(NB):
                nc.gpsimd.indirect_dma_start(
                    out=cover,
                    out_offset=bass.IndirectOffsetOnAxis(
                        ap=patch[:, b:b + 1], axis=0),
                    in_=vals[:, b:b + 1],
                    in_offset=None,
                    bounds_check=NOUT - 1,
                    oob_is_err=False)

            # Compare chunks for the rest of the row.
            dma_engines = [nc.sync, nc.scalar]
            for ci in range(NPRE, NCHUNK):
                o = wpool.tile([128, 2 * WH], mybir.dt.int16, tag="o")
                for b in range(NB):
                    # o = (iota == (label>>1) - chunk_base) * (1 or 256)
                    nc.vector.tensor_scalar(
                        out=o[:, b * WH:(b + 1) * WH], in0=iota[:, :],
                        scalar1=lab[:, b * NCHUNK + ci:b * NCHUNK + ci + 1],
                        scalar2=lab[:, NB * NCHUNK + b:NB * NCHUNK + b + 1],
                        op0=mybir.AluOpType.is_equal,
                        op1=mybir.AluOpType.mult)
                # Last chunk: split across both queues so the final
                # completion semaphores land right after the data.
                chunk_dma(dma_engines[ci % 2], o, ci, split=(ci == NCHUNK - 1))
    nc.finalize()
    return nc


def kernel(**inputs):
    from concourse.bass_utils import run_bass_kernel_spmd

    if "nc" not in _cache:
        _cache["nc"] = _build_nc()
    nc = _cache["nc"]

    lab = np.asarray(inputs["labels"]).reshape(-1).astype(np.int64)
    in_maps = []
    for i in range(N_CORES):
        shard = lab[i * RPC:(i + 1) * RPC].reshape(NB, 128)   # [b, p]
        half = shard >> 1                                     # label // 2
        # lab[p, b*NCHUNK + c] = (label >> 1) - c*WH
        # lab[p, NB*NCHUNK + b] = 1 if label even else 256
        labm = half[:, :, None] - (np.arange(NCHUNK) * WH)[None, None, :]
        labm = labm.transpose(1, 0, 2).reshape(128, NB * NCHUNK)
        scl = np.where(shard & 1, 256, 1).T                   # [p, b]
        packed = np.concatenate([labm, scl], axis=1).astype(np.float32)
        # patch[p, b] = flat pair offset (or OOB sentinel); [p, 2] = both
        # int16 values packed little-endian into one int32; [p, 3] = pad
        rowbase = (np.arange(NB)[:, None] * 128 + np.arange(128)[None, :])
        off = np.where(half < PRE_H, rowbase * VH + half, SENTINEL)  # [b, p]
        vpack = (scl[:, 0] | (scl[:, 1] << 16))[:, None]             # [p, 1]
        pad = np.zeros_like(vpack)
        pat = np.concatenate([off.T, vpack, pad], axis=1).astype(np.int32)
        in_maps.append({"lab": np.ascontiguousarray(packed),
                        "patch": np.ascontiguousarray(pat)})

    trace = bool(int(os.environ.get("BASS_KERNEL_TRACE", "0")))
    res = run_bass_kernel_spmd(nc, in_maps, list(range(N_CORES)), trace=trace)
    _cache["last_res"] = res

    outs = [np.asarray(res.results[i]["out"]).reshape(NB * 128, VH)
            for i in range(N_CORES)]
    packed = np.concatenate(outs, axis=0)                     # [ROWS, VH] i16
    return packed.view(np.uint8).astype(np.float32).reshape(B, T, V)
